# revision 11
# baseline (speedup 1.0000x reference)
"""Single-head attention (B=8, S=2048, D=384) on 8 NeuronCores.

Sharding: data-parallel over batch — core b computes batch element b
entirely, weights replicated.

Host-side marshalling (layout only, zero FLOPs): x fed pre-transposed per
core as xT [D, S]; Wv pre-transposed as WvT [D, D]; Wq/Wk fed in natural
torch [out=e, in=d] layout.

Key algebraic restructure vs v1: softmax(Q K^T) only needs
x (Wq^T Wk) x^T, so a small M = Wq^T Wk [D, D] is computed on-device
(9 matmuls) and the entire K projection (36 matmuls + evacuations) is
dropped.  GT = M^T @ xT replaces QT; scores use xT itself as the
stationary side.

Per-core dataflow (one NeuronCore, f32 in/out):
  - warm-up: a burst of dummy f32r matmuls on a zeroed SBUF tile keeps the
    PE HAM activity window busy during the input-DMA head so the real
    matmuls start at the full 2.4 GHz clock instead of 1.2 GHz.
  - M = Wq^T Wk via PE (f32r), GT[d2, s] = sum_d1 M[d1,d2] xT[d1, s],
    V natural [S, D] with two ones-columns appended -> vA [S, D+2] in bf16.
  - scores^T tile alphaT[k, q] = xT-block^T @ GT-chunk accumulated over the
    3 d2-tiles; exp() on ScalarE straight to bf16 (no max subtraction:
    |logit| << 88 so fp32 exp cannot overflow; softmax is shift-invariant).
  - out_raw[q, :D] and the softmax denominator accumulate TOGETHER via
    out_acc[q, 0:D+2] += expT[k, q-block]^T @ vA[k-block, :] (ones columns
    of vA make column D the denominator) — bf16 operands give the PV
    weight loads FWL (fast weight load), hiding them fully.
  - out[q, e] = out_raw[q, e] * (1 / out_acc[q, D]).

QK-side matmuls stay float32r (bf16 scores fail the 2e-2 gate: logit
noise ~N(0, 0.02^2)*6.5 flips softmax weights by several %).
"""

import numpy as np

import concourse.bacc as bacc
import concourse.tile as tile
from concourse import mybir
from concourse import bass_utils

P = 128          # partitions / PE tile edge
S = 2048         # sequence length per core
D = 384          # model dim
NB = 8           # batch == number of cores
DT = D // P      # 3 feature tiles
ST = S // P      # 16 sequence tiles
QC = 512         # q-column chunk (PSUM bank of f32)
NQ = S // QC     # 4 q chunks
F32 = mybir.dt.float32
F32R = mybir.dt.float32r
BF16 = mybir.dt.bfloat16

N_WARM = 5       # warm-up matmuls (N=512 f32r, ~427ns each cold)


def _build():
    nc = bacc.Bacc(
        "TRN2", target_bir_lowering=False, debug=False, enable_asserts=False
    )
    # DRAM inputs carry f32r so the direct DMA is cast-free (identical
    # 4-byte layout)
    xt = nc.dram_tensor("xt", [D, S], F32R, kind="ExternalInput").ap()
    wq = nc.dram_tensor("wq", [D, D], F32R, kind="ExternalInput").ap()
    wk = nc.dram_tensor("wk", [D, D], F32R, kind="ExternalInput").ap()
    wvt = nc.dram_tensor("wvt", [D, D], F32R, kind="ExternalInput").ap()
    out = nc.dram_tensor("out", [S, D], F32, kind="ExternalOutput").ap()

    with tile.TileContext(nc) as tc:
        with (
            tc.tile_pool(name="const", bufs=1) as const_pool,
            tc.tile_pool(name="big", bufs=1) as big,
            tc.tile_pool(name="expool", bufs=4) as ex_pool,
            tc.tile_pool(name="obpool", bufs=3) as ob_pool,
            tc.tile_pool(name="smalls", bufs=4) as small_pool,
            tc.tile_pool(name="ps_stage", bufs=4, space="PSUM") as ps_stage,
            tc.tile_pool(name="ps_acc", bufs=4, space="PSUM") as ps_acc,
        ):
            ones_c = const_pool.tile([P, 2], BF16, tag="ones", name="ones_c")
            nc.vector.memset(ones_c, 1.0)
            warm_sb = const_pool.tile([P, QC], F32, tag="warm", name="warm")
            nc.vector.memset(warm_sb, 0.0)
            warm_r = warm_sb.bitcast(F32R)

            # ---- PE warm-up: keep the HAM activity monitor busy while the
            # input DMAs stream in, so real matmuls start at 2.4 GHz.
            for wi in range(N_WARM):
                pw = ps_stage.tile([P, QC], F32, tag="ps1", name="pw")
                nc.tensor.matmul(
                    pw, warm_r[:, 0:P], warm_r, start=True, stop=True
                )

            # Persistent per-core operands.
            xT = big.tile([P, DT, S], F32R, tag="xT", name="xT")
            gT = big.tile([P, DT, S], F32R, tag="gT", name="gT")
            # +2 ones columns (even free size for the PE; col D and D+1 = 1)
            vA = big.tile([P, ST, D + 2], BF16, tag="vA", name="vA")
            wqN = big.tile([P, DT, D], F32R, tag="wqN", name="wqN")
            wkN = big.tile([P, DT, D], F32R, tag="wkN", name="wkN")
            wvT = big.tile([P, DT, D], F32R, tag="wvT", name="wvT")
            mSB = big.tile([P, DT, D], F32R, tag="mSB", name="mSB")

            # ---- load operands ------------------------------------------
            # sync queue, ordered as the PE consumes: wv, wq, x cols
            # 512:2048.  gpsimd rings carry x cols 0:512 (split so the
            # first V-projections start as soon as wv lands) then wk.
            def dma_w(engine, w_dram, wT):
                for dt_ in range(DT):
                    engine.dma_start(
                        out=wT[:, dt_, :],
                        in_=w_dram[dt_ * P:(dt_ + 1) * P, :],
                    )

            def dma_x(engine, lo, hi):
                for dt_ in range(DT):
                    engine.dma_start(
                        out=xT[:, dt_, lo:hi],
                        in_=xt[dt_ * P:(dt_ + 1) * P, lo:hi],
                    )

            # Critical set spread across all three rings so it gets the
            # full aggregate HBM bandwidth: x0 on gpsimd, wv on sync,
            # wq/wk on scalar.  x1-3 are dispatched right away but their
            # ring entry is held back by a WAW dependency on a one-element
            # DVE write (emitted after the V0-3 evacuations below), so the
            # critical loads never share bandwidth with them.
            dma_x(nc.gpsimd, 0, P)
            dma_x(nc.gpsimd, P, QC)
            dma_w(nc.sync, wvt, wvT)
            dma_w(nc.scalar, wq, wqN)
            dma_w(nc.scalar, wk, wkN)

            def dma_x_rest():
                dma_x(nc.sync, QC, 2 * QC)
                dma_x(nc.scalar, 2 * QC, 3 * QC)
                dma_x(nc.sync, 3 * QC, 4 * QC)

            # ---- projections ---------------------------------------------
            # Rotate staging across BOTH psum pools: the 4 accumulator
            # banks are idle during this phase, and 8 rotating banks let
            # the PE run ahead of the DVE drain.
            _proj_n = [0]

            def proj_tile(n=QC):
                _proj_n[0] += 1
                if _proj_n[0] % 2:
                    return ps_stage.tile([P, n], F32, tag="ps1", name="pj")
                return ps_acc.tile([P, n], F32, tag="acc", name="pj")

            def proj_tile_stage(n=QC):
                # attention holds all 4 ps_acc banks for a whole chunk, so
                # projections interleaved into the attention stream must
                # draw only from the ps_stage rotation (shared with pa).
                return ps_stage.tile([P, n], F32, tag="ps1", name="pj")

            def project_v(st, tile_fn=None):
                # V natural: V[s, e] = sum_d xT[d, s] * WvT[d, e]
                pv = (tile_fn or proj_tile)()
                for dt_ in range(DT):
                    nc.tensor.matmul(
                        pv[:, 0:D],
                        xT[:, dt_, st * P:(st + 1) * P],
                        wvT[:, dt_, :],
                        start=(dt_ == 0),
                        stop=(dt_ == DT - 1),
                    )
                nc.vector.tensor_copy(vA[:, st, 0:D], pv[:, 0:D])
                nc.vector.tensor_copy(vA[:, st, D:D + 2], ones_c)

            def project_m():
                # M[d1, d2] = sum_e Wq[e, d1] * Wk[e, d2]
                for d1t in range(DT):
                    pm = proj_tile()
                    for et in range(DT):
                        nc.tensor.matmul(
                            pm[:, 0:D],
                            wqN[:, et, d1t * P:(d1t + 1) * P],
                            wkN[:, et, :],
                            start=(et == 0),
                            stop=(et == DT - 1),
                        )
                    nc.vector.tensor_copy(mSB[:, d1t, :], pm[:, 0:D])

            def project_g(qc, tile_fn=None):
                # GT[d2, s] = sum_d1 M[d1, d2] * xT[d1, s]
                for d2t in range(DT):
                    pp = (tile_fn or proj_tile)()
                    for d1t in range(DT):
                        nc.tensor.matmul(
                            pp,
                            mSB[:, d1t, d2t * P:(d2t + 1) * P],
                            xT[:, d1t, qc * QC:(qc + 1) * QC],
                            start=(d1t == 0),
                            stop=(d1t == DT - 1),
                        )
                    nc.vector.tensor_copy(
                        gT[:, d2t, qc * QC:(qc + 1) * QC], pp
                    )

            # Minimal pre-attention projections: V0-3 (wv + x0), M (wq/wk),
            # GT0 — attention chunk 0 starts right after; the remaining V
            # rows and GT chunks are interleaved into the attention stream
            # below, hidden under its matmuls while x1-3 stream in.
            def filler():
                nc.tensor.matmul(
                    proj_tile_stage(), warm_r[:, 0:P], warm_r,
                    start=True, stop=True,
                )

            for st in range(4):
                project_v(st)
            # release the x1-3 loads: these one-element writes land on the
            # DVE stream after the vA0-3 evacuations, and the x-chunk DMAs
            # wait on them (WAW) before entering the rings.
            for qc in range(1, NQ):
                nc.vector.tensor_copy(
                    xT[:, 0, qc * QC:qc * QC + 1], ones_c[:, 0:1]
                )
            dma_x_rest()
            filler()
            project_m()
            project_g(0)

            # ---- attention, one 512-wide q chunk at a time ----------------
            for c in range(NQ):
                accs = [
                    ps_acc.tile([P, D + 2], F32, tag="acc", name="acc")
                    for _ in range(4)
                ]

                def emit_pv(kt_i, ex):
                    for qs in range(4):
                        nc.tensor.matmul(
                            accs[qs],
                            ex[:, qs * P:(qs + 1) * P],
                            vA[:, kt_i, :],
                            start=(kt_i == 0),
                            stop=(kt_i == ST - 1),
                        )

                pending = []
                for kt_i in range(ST):
                    # deferred projections, hidden under attention matmuls
                    if c == 0:
                        if kt_i == 3:
                            for st in range(4, 8):
                                project_v(st, proj_tile_stage)
                        elif kt_i == 5:
                            for st in range(8, 12):
                                project_v(st, proj_tile_stage)
                        elif kt_i == 8:
                            project_g(1, proj_tile_stage)
                        elif kt_i == 10:
                            for st in range(12, 16):
                                project_v(st, proj_tile_stage)
                        elif kt_i == 13:
                            project_g(2, proj_tile_stage)
                    elif c == 1 and kt_i == 2:
                        project_g(3, proj_tile_stage)
                    pa = ps_stage.tile([P, QC], F32, tag="ps1", name="pa")
                    for et in range(DT):
                        nc.tensor.matmul(
                            pa,
                            xT[:, et, kt_i * P:(kt_i + 1) * P],
                            gT[:, et, c * QC:(c + 1) * QC],
                            start=(et == 0),
                            stop=(et == DT - 1),
                        )
                    ex = ex_pool.tile([P, QC], BF16, tag="ex", name="ex")
                    nc.scalar.activation(
                        ex, pa, mybir.ActivationFunctionType.Exp
                    )
                    # software-pipeline PV two k-tiles behind the QK+exp so
                    # the PE never waits on a just-issued exp
                    pending.append((kt_i, ex))
                    if len(pending) > 2:
                        emit_pv(*pending.pop(0))
                for item in pending:
                    emit_pv(*item)

                # epilogue split across DVE and ACT so the tail chain halves;
                # all reciprocals first so the ACT-side muls never wait on a
                # reciprocal queued behind a DVE mul
                recs = []
                for qs in range(4):
                    rec = small_pool.tile([P, 1], F32, tag="rec", name="rec")
                    nc.vector.reciprocal(rec, accs[qs][:, D:D + 1])
                    recs.append(rec)
                for qs in range(4):
                    ob = ob_pool.tile([P, D], F32, tag="ob", name="ob")
                    qt_row = (c * 4 + qs) * P
                    if qs % 2:
                        nc.scalar.activation(
                            ob,
                            accs[qs][:, 0:D],
                            mybir.ActivationFunctionType.Copy,
                            scale=recs[qs],
                        )
                        nc.scalar.dma_start(
                            out=out[qt_row:qt_row + P, :], in_=ob
                        )
                    else:
                        nc.vector.tensor_scalar_mul(
                            ob, accs[qs][:, 0:D], recs[qs]
                        )
                        nc.sync.dma_start(
                            out=out[qt_row:qt_row + P, :], in_=ob
                        )

    nc.compile()
    return nc


_NC = None
_FAST = None


def _get_nc():
    global _NC
    if _NC is None:
        _NC = _build()
    return _NC


def _fast_runner():
    """Build (once) a jitted shard_map callable over the 8 cores."""
    global _FAST
    if _FAST is not None:
        return _FAST
    import jax
    from jax.experimental.shard_map import shard_map
    from jax.sharding import Mesh, PartitionSpec

    from concourse import bass2jax

    nc = _get_nc()
    bass2jax.install_neuronx_cc_hook()

    in_names = ["xt", "wq", "wk", "wvt"]
    out_aval = jax.core.ShapedArray((S, D), np.float32)

    def _body(*args):
        operands = list(args)
        operands.append(bass2jax.partition_id_tensor())
        outs = bass2jax._bass_exec_p.bind(
            *operands,
            out_avals=(out_aval,),
            in_names=tuple(in_names) + ("out", "partition_id"),
            out_names=("out",),
            lowering_input_output_aliases=(),
            sim_require_finite=True,
            sim_require_nnan=True,
            nc=nc,
        )
        return tuple(outs)

    devices = jax.devices()[:NB]
    mesh = Mesh(np.asarray(devices), ("core",))
    n_in = len(in_names) + 1  # + donated zero output
    fn = jax.jit(
        shard_map(
            _body,
            mesh=mesh,
            in_specs=(PartitionSpec("core"),) * n_in,
            out_specs=(PartitionSpec("core"),),
            check_rep=False,
        ),
        donate_argnums=(n_in - 1,),
        keep_unused=True,
    )
    _FAST = fn
    return fn


def _marshal(att_input, Wq, Wk, Wv):
    att_input = np.asarray(att_input, dtype=np.float32)
    # pre-transposed per-core x; Wv transposed; Wq/Wk natural (layout only)
    xts = np.ascontiguousarray(att_input.transpose(0, 2, 1))  # [NB, D, S]
    wq = np.ascontiguousarray(np.asarray(Wq, dtype=np.float32))
    wk = np.ascontiguousarray(np.asarray(Wk, dtype=np.float32))
    wvt = np.ascontiguousarray(np.asarray(Wv, dtype=np.float32).T)
    return xts, (wq, wk, wvt)


def run(att_input, Wq, Wk, Wv, trace=False):
    xts, wts = _marshal(att_input, Wq, Wk, Wv)
    if trace:
        in_maps = [
            {"xt": xts[b], "wq": wts[0], "wk": wts[1], "wvt": wts[2]}
            for b in range(NB)
        ]
        res = bass_utils.run_bass_kernel_spmd(
            _get_nc(), in_maps, core_ids=list(range(NB)), trace=True
        )
        out = np.stack([res.results[b]["out"] for b in range(NB)], axis=0)
        return out.astype(np.float32, copy=False), res

    try:
        fn = _fast_runner()
        xs = xts.reshape(NB * D, S)
        ws = [np.concatenate([w] * NB, axis=0) for w in wts]
        zeros = np.zeros((NB * S, D), np.float32)
        (out,) = fn(xs, *ws, zeros)
        out = np.asarray(out)
    except Exception:
        # fallback: the stock SPMD runner (re-jits per call, same NEFF)
        in_maps = [
            {"xt": xts[b], "wq": wts[0], "wk": wts[1], "wvt": wts[2]}
            for b in range(NB)
        ]
        res = bass_utils.run_bass_kernel_spmd(
            _get_nc(), in_maps, core_ids=list(range(NB))
        )
        out = np.stack([res.results[b]["out"] for b in range(NB)], axis=0)
    return out.reshape(NB, S, D).astype(np.float32, copy=False), None


def kernel(att_input, Wq, Wk, Wv):
    out, _ = run(att_input, Wq, Wk, Wv)
    return out


# revision 12
# speedup vs baseline: 1.1555x; 1.1555x over previous
"""Single-head attention (B=8, S=2048, D=384) on 8 NeuronCores.

Sharding: data-parallel over batch — core b computes batch element b
entirely, weights replicated.

Host-side marshalling (layout only, zero FLOPs): x fed pre-transposed per
core as xT [D, S]; Wv pre-transposed as WvT [D, D]; Wq/Wk fed in natural
torch [out=e, in=d] layout.

Key algebraic restructure vs v1: softmax(Q K^T) only needs
x (Wq^T Wk) x^T, so a small M = Wq^T Wk [D, D] is computed on-device
(9 matmuls) and the entire K projection (36 matmuls + evacuations) is
dropped.  GT = M^T @ xT replaces QT; scores use xT itself as the
stationary side.

Per-core dataflow (one NeuronCore, f32 in/out):
  - warm-up: a burst of dummy f32r matmuls on a zeroed SBUF tile keeps the
    PE HAM activity window busy during the input-DMA head so the real
    matmuls start at the full 2.4 GHz clock instead of 1.2 GHz.
  - M = Wq^T Wk via PE (f32r), GT[d2, s] = sum_d1 M[d1,d2] xT[d1, s],
    V natural [S, D] with two ones-columns appended -> vA [S, D+2] in bf16.
  - scores^T tile alphaT[k, q] = xT-block^T @ GT-chunk accumulated over the
    3 d2-tiles; exp() on ScalarE straight to bf16 (no max subtraction:
    |logit| << 88 so fp32 exp cannot overflow; softmax is shift-invariant).
  - out_raw[q, :D] and the softmax denominator accumulate TOGETHER via
    out_acc[q, 0:D+2] += expT[k, q-block]^T @ vA[k-block, :] (ones columns
    of vA make column D the denominator) — bf16 operands give the PV
    weight loads FWL (fast weight load), hiding them fully.
  - out[q, e] = out_raw[q, e] * (1 / out_acc[q, D]).

QK-side matmuls stay float32r (bf16 scores fail the 2e-2 gate: logit
noise ~N(0, 0.02^2)*6.5 flips softmax weights by several %).
"""

import numpy as np

import concourse.bacc as bacc
import concourse.tile as tile
from concourse import mybir
from concourse import bass_utils

P = 128          # partitions / PE tile edge
S = 2048         # sequence length per core
D = 384          # model dim
NB = 8           # batch == number of cores
DT = D // P      # 3 feature tiles
ST = S // P      # 16 sequence tiles
QC = 512         # q-column chunk (PSUM bank of f32)
NQ = S // QC     # 4 q chunks
F32 = mybir.dt.float32
F32R = mybir.dt.float32r
BF16 = mybir.dt.bfloat16

N_WARM = 5       # warm-up matmuls (N=512 f32r, ~427ns each cold)


def _build():
    nc = bacc.Bacc(
        "TRN2", target_bir_lowering=False, debug=False, enable_asserts=False
    )
    # DRAM inputs carry f32r so the direct DMA is cast-free (identical
    # 4-byte layout)
    xt = nc.dram_tensor("xt", [D, S], F32R, kind="ExternalInput").ap()
    wq = nc.dram_tensor("wq", [D, D], F32R, kind="ExternalInput").ap()
    wk = nc.dram_tensor("wk", [D, D], F32R, kind="ExternalInput").ap()
    wvt = nc.dram_tensor("wvt", [D, D], F32R, kind="ExternalInput").ap()
    out = nc.dram_tensor("out", [S, D], F32, kind="ExternalOutput").ap()

    with tile.TileContext(nc) as tc:
        with (
            tc.tile_pool(name="const", bufs=1) as const_pool,
            tc.tile_pool(name="big", bufs=1) as big,
            tc.tile_pool(name="expool", bufs=4) as ex_pool,
            tc.tile_pool(name="obpool", bufs=3) as ob_pool,
            tc.tile_pool(name="smalls", bufs=4) as small_pool,
            tc.tile_pool(name="ps_stage", bufs=4, space="PSUM") as ps_stage,
            tc.tile_pool(name="ps_acc", bufs=4, space="PSUM") as ps_acc,
        ):
            ones_c = const_pool.tile([P, 2], BF16, tag="ones", name="ones_c")
            nc.vector.memset(ones_c, 1.0)
            warm_sb = const_pool.tile([P, QC], F32, tag="warm", name="warm")
            nc.vector.memset(warm_sb, 0.0)
            warm_r = warm_sb.bitcast(F32R)

            # ---- PE warm-up: keep the HAM activity monitor busy while the
            # input DMAs stream in, so real matmuls start at 2.4 GHz.
            for wi in range(N_WARM):
                pw = ps_stage.tile([P, QC], F32, tag="ps1", name="pw")
                nc.tensor.matmul(
                    pw, warm_r[:, 0:P], warm_r, start=True, stop=True
                )

            # Persistent per-core operands.
            xT = big.tile([P, DT, S], F32R, tag="xT", name="xT")
            gT = big.tile([P, DT, S], F32R, tag="gT", name="gT")
            # +2 ones columns (even free size for the PE; col D and D+1 = 1)
            vA = big.tile([P, ST, D + 2], BF16, tag="vA", name="vA")
            wqN = big.tile([P, DT, D], F32R, tag="wqN", name="wqN")
            wkN = big.tile([P, DT, D], F32R, tag="wkN", name="wkN")
            wvT = big.tile([P, DT, D], F32R, tag="wvT", name="wvT")
            mSB = big.tile([P, DT, D], F32R, tag="mSB", name="mSB")

            # ---- load operands ------------------------------------------
            # sync queue, ordered as the PE consumes: wv, wq, x cols
            # 512:2048.  gpsimd rings carry x cols 0:512 (split so the
            # first V-projections start as soon as wv lands) then wk.
            def dma_w(engine, w_dram, wT):
                for dt_ in range(DT):
                    engine.dma_start(
                        out=wT[:, dt_, :],
                        in_=w_dram[dt_ * P:(dt_ + 1) * P, :],
                    )

            def dma_x(engine, lo, hi):
                for dt_ in range(DT):
                    engine.dma_start(
                        out=xT[:, dt_, lo:hi],
                        in_=xt[dt_ * P:(dt_ + 1) * P, lo:hi],
                    )

            # Critical set spread across all three rings so it gets the
            # full aggregate HBM bandwidth: x0 on gpsimd, wv on sync,
            # wq/wk on scalar.  x1-3 are dispatched right away but their
            # ring entry is held back by a WAW dependency on a one-element
            # DVE write (emitted after the V0-3 evacuations below), so the
            # critical loads never share bandwidth with them.
            dma_x(nc.gpsimd, 0, P)
            dma_x(nc.gpsimd, P, QC)
            dma_w(nc.sync, wvt, wvT)
            dma_w(nc.sync, wk, wkN)
            dma_w(nc.scalar, wq, wqN)

            def dma_x_rest():
                dma_x(nc.sync, QC, 2 * QC)
                dma_x(nc.scalar, 2 * QC, 3 * QC)
                dma_x(nc.sync, 3 * QC, 4 * QC)

            # ---- projections ---------------------------------------------
            # Rotate staging across BOTH psum pools: the 4 accumulator
            # banks are idle during this phase, and 8 rotating banks let
            # the PE run ahead of the DVE drain.
            _proj_n = [0]

            def proj_tile(n=QC):
                _proj_n[0] += 1
                if _proj_n[0] % 2:
                    return ps_stage.tile([P, n], F32, tag="ps1", name="pj")
                return ps_acc.tile([P, n], F32, tag="acc", name="pj")

            def proj_tile_stage(n=QC):
                # attention holds all 4 ps_acc banks for a whole chunk, so
                # projections interleaved into the attention stream must
                # draw only from the ps_stage rotation (shared with pa).
                return ps_stage.tile([P, n], F32, tag="ps1", name="pj")

            def project_v(st, tile_fn=None):
                # V natural: V[s, e] = sum_d xT[d, s] * WvT[d, e]
                pv = (tile_fn or proj_tile)()
                for dt_ in range(DT):
                    nc.tensor.matmul(
                        pv[:, 0:D],
                        xT[:, dt_, st * P:(st + 1) * P],
                        wvT[:, dt_, :],
                        start=(dt_ == 0),
                        stop=(dt_ == DT - 1),
                    )
                nc.vector.tensor_copy(vA[:, st, 0:D], pv[:, 0:D])
                nc.vector.tensor_copy(vA[:, st, D:D + 2], ones_c)

            def project_m():
                # M[d1, d2] = sum_e Wq[e, d1] * Wk[e, d2]
                for d1t in range(DT):
                    pm = proj_tile()
                    for et in range(DT):
                        nc.tensor.matmul(
                            pm[:, 0:D],
                            wqN[:, et, d1t * P:(d1t + 1) * P],
                            wkN[:, et, :],
                            start=(et == 0),
                            stop=(et == DT - 1),
                        )
                    nc.vector.tensor_copy(mSB[:, d1t, :], pm[:, 0:D])

            def project_g(qc, tile_fn=None):
                # GT[d2, s] = sum_d1 M[d1, d2] * xT[d1, s]
                for d2t in range(DT):
                    pp = (tile_fn or proj_tile)()
                    for d1t in range(DT):
                        nc.tensor.matmul(
                            pp,
                            mSB[:, d1t, d2t * P:(d2t + 1) * P],
                            xT[:, d1t, qc * QC:(qc + 1) * QC],
                            start=(d1t == 0),
                            stop=(d1t == DT - 1),
                        )
                    nc.vector.tensor_copy(
                        gT[:, d2t, qc * QC:(qc + 1) * QC], pp
                    )

            # Minimal pre-attention projections: V0-3 (wv + x0), M (wq/wk),
            # GT0 — attention chunk 0 starts right after; the remaining V
            # rows and GT chunks are interleaved into the attention stream
            # below, hidden under its matmuls while x1-3 stream in.
            def filler():
                nc.tensor.matmul(
                    proj_tile_stage(), warm_r[:, 0:P], warm_r,
                    start=True, stop=True,
                )

            for st in range(4):
                project_v(st)
            # release the x1-3 loads: these one-element writes land on the
            # DVE stream after the vA0-3 evacuations, and the x-chunk DMAs
            # wait on them (WAW) before entering the rings.
            for qc in range(1, NQ):
                nc.vector.tensor_copy(
                    xT[:, 0, qc * QC:qc * QC + 1], ones_c[:, 0:1]
                )
            dma_x_rest()
            for _ in range(3):
                filler()
            project_m()
            filler()
            filler()
            project_g(0)

            # ---- attention, one 512-wide q chunk at a time ----------------
            for c in range(NQ):
                accs = [
                    ps_acc.tile([P, D + 2], F32, tag="acc", name="acc")
                    for _ in range(4)
                ]

                def emit_pv(kt_i, ex):
                    for qs in range(4):
                        nc.tensor.matmul(
                            accs[qs],
                            ex[:, qs * P:(qs + 1) * P],
                            vA[:, kt_i, :],
                            start=(kt_i == 0),
                            stop=(kt_i == ST - 1),
                        )

                pending = []
                for kt_i in range(ST):
                    # deferred projections, hidden under attention matmuls
                    if c == 0:
                        if kt_i == 3:
                            for st in range(4, 8):
                                project_v(st, proj_tile_stage)
                        elif kt_i == 5:
                            for st in range(8, 12):
                                project_v(st, proj_tile_stage)
                        elif kt_i == 8:
                            project_g(1, proj_tile_stage)
                        elif kt_i == 10:
                            for st in range(12, 16):
                                project_v(st, proj_tile_stage)
                        elif kt_i == 13:
                            project_g(2, proj_tile_stage)
                    elif c == 1 and kt_i == 2:
                        project_g(3, proj_tile_stage)
                    pa = ps_stage.tile([P, QC], F32, tag="ps1", name="pa")
                    for et in range(DT):
                        nc.tensor.matmul(
                            pa,
                            xT[:, et, kt_i * P:(kt_i + 1) * P],
                            gT[:, et, c * QC:(c + 1) * QC],
                            start=(et == 0),
                            stop=(et == DT - 1),
                        )
                    ex = ex_pool.tile([P, QC], BF16, tag="ex", name="ex")
                    nc.scalar.activation(
                        ex, pa, mybir.ActivationFunctionType.Exp
                    )
                    # software-pipeline PV two k-tiles behind the QK+exp so
                    # the PE never waits on a just-issued exp
                    pending.append((kt_i, ex))
                    if len(pending) > 2:
                        emit_pv(*pending.pop(0))
                for item in pending:
                    emit_pv(*item)

                # epilogue split across DVE and ACT so the tail chain halves;
                # all reciprocals first so the ACT-side muls never wait on a
                # reciprocal queued behind a DVE mul
                recs = []
                for qs in range(4):
                    rec = small_pool.tile([P, 1], F32, tag="rec", name="rec")
                    nc.vector.reciprocal(rec, accs[qs][:, D:D + 1])
                    recs.append(rec)
                for qs in range(4):
                    ob = ob_pool.tile([P, D], F32, tag="ob", name="ob")
                    qt_row = (c * 4 + qs) * P
                    if qs % 2:
                        nc.scalar.activation(
                            ob,
                            accs[qs][:, 0:D],
                            mybir.ActivationFunctionType.Copy,
                            scale=recs[qs],
                        )
                        nc.scalar.dma_start(
                            out=out[qt_row:qt_row + P, :], in_=ob
                        )
                    else:
                        nc.vector.tensor_scalar_mul(
                            ob, accs[qs][:, 0:D], recs[qs]
                        )
                        nc.sync.dma_start(
                            out=out[qt_row:qt_row + P, :], in_=ob
                        )

    nc.compile()
    return nc


_NC = None
_FAST = None


def _get_nc():
    global _NC
    if _NC is None:
        _NC = _build()
    return _NC


def _fast_runner():
    """Build (once) a jitted shard_map callable over the 8 cores."""
    global _FAST
    if _FAST is not None:
        return _FAST
    import jax
    from jax.experimental.shard_map import shard_map
    from jax.sharding import Mesh, PartitionSpec

    from concourse import bass2jax

    nc = _get_nc()
    bass2jax.install_neuronx_cc_hook()

    in_names = ["xt", "wq", "wk", "wvt"]
    out_aval = jax.core.ShapedArray((S, D), np.float32)

    def _body(*args):
        operands = list(args)
        operands.append(bass2jax.partition_id_tensor())
        outs = bass2jax._bass_exec_p.bind(
            *operands,
            out_avals=(out_aval,),
            in_names=tuple(in_names) + ("out", "partition_id"),
            out_names=("out",),
            lowering_input_output_aliases=(),
            sim_require_finite=True,
            sim_require_nnan=True,
            nc=nc,
        )
        return tuple(outs)

    devices = jax.devices()[:NB]
    mesh = Mesh(np.asarray(devices), ("core",))
    n_in = len(in_names) + 1  # + donated zero output
    fn = jax.jit(
        shard_map(
            _body,
            mesh=mesh,
            in_specs=(PartitionSpec("core"),) * n_in,
            out_specs=(PartitionSpec("core"),),
            check_rep=False,
        ),
        donate_argnums=(n_in - 1,),
        keep_unused=True,
    )
    _FAST = fn
    return fn


def _marshal(att_input, Wq, Wk, Wv):
    att_input = np.asarray(att_input, dtype=np.float32)
    # pre-transposed per-core x; Wv transposed; Wq/Wk natural (layout only)
    xts = np.ascontiguousarray(att_input.transpose(0, 2, 1))  # [NB, D, S]
    wq = np.ascontiguousarray(np.asarray(Wq, dtype=np.float32))
    wk = np.ascontiguousarray(np.asarray(Wk, dtype=np.float32))
    wvt = np.ascontiguousarray(np.asarray(Wv, dtype=np.float32).T)
    return xts, (wq, wk, wvt)


def run(att_input, Wq, Wk, Wv, trace=False):
    xts, wts = _marshal(att_input, Wq, Wk, Wv)
    if trace:
        in_maps = [
            {"xt": xts[b], "wq": wts[0], "wk": wts[1], "wvt": wts[2]}
            for b in range(NB)
        ]
        res = bass_utils.run_bass_kernel_spmd(
            _get_nc(), in_maps, core_ids=list(range(NB)), trace=True
        )
        out = np.stack([res.results[b]["out"] for b in range(NB)], axis=0)
        return out.astype(np.float32, copy=False), res

    try:
        fn = _fast_runner()
        xs = xts.reshape(NB * D, S)
        ws = [np.concatenate([w] * NB, axis=0) for w in wts]
        zeros = np.zeros((NB * S, D), np.float32)
        (out,) = fn(xs, *ws, zeros)
        out = np.asarray(out)
    except Exception:
        # fallback: the stock SPMD runner (re-jits per call, same NEFF)
        in_maps = [
            {"xt": xts[b], "wq": wts[0], "wk": wts[1], "wvt": wts[2]}
            for b in range(NB)
        ]
        res = bass_utils.run_bass_kernel_spmd(
            _get_nc(), in_maps, core_ids=list(range(NB))
        )
        out = np.stack([res.results[b]["out"] for b in range(NB)], axis=0)
    return out.reshape(NB, S, D).astype(np.float32, copy=False), None


def kernel(att_input, Wq, Wk, Wv):
    out, _ = run(att_input, Wq, Wk, Wv)
    return out


# revision 14
# speedup vs baseline: 1.1777x; 1.0192x over previous
"""Single-head attention (B=8, S=2048, D=384) on 8 NeuronCores.

Sharding: data-parallel over batch — core b computes batch element b
entirely, weights replicated.

Host-side marshalling (layout only, zero FLOPs): x fed pre-transposed per
core as xT [D, S]; Wv pre-transposed as WvT [D, D]; Wq/Wk fed in natural
torch [out=e, in=d] layout.

Key algebraic restructure vs v1: softmax(Q K^T) only needs
x (Wq^T Wk) x^T, so a small M = Wq^T Wk [D, D] is computed on-device
(9 matmuls) and the entire K projection (36 matmuls + evacuations) is
dropped.  GT = M^T @ xT replaces QT; scores use xT itself as the
stationary side.

Per-core dataflow (one NeuronCore, f32 in/out):
  - warm-up: a burst of dummy f32r matmuls on a zeroed SBUF tile keeps the
    PE HAM activity window busy during the input-DMA head so the real
    matmuls start at the full 2.4 GHz clock instead of 1.2 GHz.
  - M = Wq^T Wk via PE (f32r), GT[d2, s] = sum_d1 M[d1,d2] xT[d1, s],
    V natural [S, D] with two ones-columns appended -> vA [S, D+2] in bf16.
  - scores^T tile alphaT[k, q] = xT-block^T @ GT-chunk accumulated over the
    3 d2-tiles; exp() on ScalarE straight to bf16 (no max subtraction:
    |logit| << 88 so fp32 exp cannot overflow; softmax is shift-invariant).
  - out_raw[q, :D] and the softmax denominator accumulate TOGETHER via
    out_acc[q, 0:D+2] += expT[k, q-block]^T @ vA[k-block, :] (ones columns
    of vA make column D the denominator) — bf16 operands give the PV
    weight loads FWL (fast weight load), hiding them fully.
  - out[q, e] = out_raw[q, e] * (1 / out_acc[q, D]).

QK-side matmuls stay float32r (bf16 scores fail the 2e-2 gate: logit
noise ~N(0, 0.02^2)*6.5 flips softmax weights by several %).
"""

import numpy as np

import concourse.bacc as bacc
import concourse.tile as tile
from concourse import mybir
from concourse import bass_utils

P = 128          # partitions / PE tile edge
S = 2048         # sequence length per core
D = 384          # model dim
NB = 8           # batch == number of cores
DT = D // P      # 3 feature tiles
ST = S // P      # 16 sequence tiles
QC = 512         # q-column chunk (PSUM bank of f32)
NQ = S // QC     # 4 q chunks
F32 = mybir.dt.float32
F32R = mybir.dt.float32r
BF16 = mybir.dt.bfloat16

N_WARM = 5       # warm-up matmuls (N=512 f32r, ~427ns each cold)


def _build():
    nc = bacc.Bacc(
        "TRN2", target_bir_lowering=False, debug=False, enable_asserts=False
    )
    # DRAM inputs carry f32r so the direct DMA is cast-free (identical
    # 4-byte layout)
    xt = nc.dram_tensor("xt", [D, S], F32R, kind="ExternalInput").ap()
    wq = nc.dram_tensor("wq", [D, D], F32R, kind="ExternalInput").ap()
    wk = nc.dram_tensor("wk", [D, D], F32R, kind="ExternalInput").ap()
    wvt = nc.dram_tensor("wvt", [D, D], F32R, kind="ExternalInput").ap()
    out = nc.dram_tensor("out", [S, D], F32, kind="ExternalOutput").ap()

    with tile.TileContext(nc) as tc:
        with (
            tc.tile_pool(name="const", bufs=1) as const_pool,
            tc.tile_pool(name="big", bufs=1) as big,
            tc.tile_pool(name="expool", bufs=4) as ex_pool,
            tc.tile_pool(name="obpool", bufs=3) as ob_pool,
            tc.tile_pool(name="smalls", bufs=4) as small_pool,
            tc.tile_pool(name="ps_stage", bufs=4, space="PSUM") as ps_stage,
            tc.tile_pool(name="ps_acc", bufs=4, space="PSUM") as ps_acc,
        ):
            ones_c = const_pool.tile([P, 2], BF16, tag="ones", name="ones_c")
            nc.vector.memset(ones_c, 1.0)
            warm_sb = const_pool.tile([P, QC], F32, tag="warm", name="warm")
            nc.vector.memset(warm_sb, 0.0)
            warm_r = warm_sb.bitcast(F32R)

            # ---- PE warm-up: keep the HAM activity monitor busy while the
            # input DMAs stream in, so real matmuls start at 2.4 GHz.
            # All warm-ups write ONE tile: drawing a fresh rotation tile
            # per warm-up stalls after bufs(4) iterations (reader-less
            # PSUM tiles release late), which left the head cold in every
            # earlier variant.  A WAW chain on a single tile keeps them
            # back-to-back while real work interleaves as its DMAs land.
            pw = ps_stage.tile([P, QC], F32, tag="ps1", name="pw")
            for wi in range(N_WARM):
                nc.tensor.matmul(
                    pw, warm_r[:, 0:P], warm_r, start=True, stop=True
                )

            # Persistent per-core operands.
            xT = big.tile([P, DT, S], F32R, tag="xT", name="xT")
            gT = big.tile([P, DT, S], F32R, tag="gT", name="gT")
            # +2 ones columns (even free size for the PE; col D and D+1 = 1)
            vA = big.tile([P, ST, D + 2], BF16, tag="vA", name="vA")
            wqN = big.tile([P, DT, D], F32R, tag="wqN", name="wqN")
            wkN = big.tile([P, DT, D], F32R, tag="wkN", name="wkN")
            wvT = big.tile([P, DT, D], F32R, tag="wvT", name="wvT")
            mSB = big.tile([P, DT, D], F32R, tag="mSB", name="mSB")

            # ---- load operands ------------------------------------------
            # sync queue, ordered as the PE consumes: wv, wq, x cols
            # 512:2048.  gpsimd rings carry x cols 0:512 (split so the
            # first V-projections start as soon as wv lands) then wk.
            def dma_w(engine, w_dram, wT):
                for dt_ in range(DT):
                    engine.dma_start(
                        out=wT[:, dt_, :],
                        in_=w_dram[dt_ * P:(dt_ + 1) * P, :],
                    )

            def dma_x(engine, lo, hi):
                for dt_ in range(DT):
                    engine.dma_start(
                        out=xT[:, dt_, lo:hi],
                        in_=xt[dt_ * P:(dt_ + 1) * P, lo:hi],
                    )

            # Critical set spread across all three rings so it gets the
            # full aggregate HBM bandwidth: x0 on gpsimd, wv on sync,
            # wq/wk on scalar.  x1-3 are dispatched right away but their
            # ring entry is held back by a WAW dependency on a one-element
            # DVE write (emitted after the V0-3 evacuations below), so the
            # critical loads never share bandwidth with them.
            dma_x(nc.gpsimd, 0, P)
            dma_x(nc.gpsimd, P, QC)
            dma_w(nc.sync, wvt, wvT)
            dma_w(nc.scalar, wq, wqN)
            dma_w(nc.scalar, wk, wkN)

            def dma_x_rest():
                dma_x(nc.sync, QC, 2 * QC)
                dma_x(nc.scalar, 2 * QC, 3 * QC)
                dma_x(nc.sync, 3 * QC, 4 * QC)

            # ---- projections ---------------------------------------------
            # Rotate staging across BOTH psum pools: the 4 accumulator
            # banks are idle during this phase, and 8 rotating banks let
            # the PE run ahead of the DVE drain.
            _proj_n = [0]

            def proj_tile(n=QC):
                _proj_n[0] += 1
                if _proj_n[0] % 2:
                    return ps_stage.tile([P, n], F32, tag="ps1", name="pj")
                return ps_acc.tile([P, n], F32, tag="acc", name="pj")

            def proj_tile_stage(n=QC):
                # attention holds all 4 ps_acc banks for a whole chunk, so
                # projections interleaved into the attention stream must
                # draw only from the ps_stage rotation (shared with pa).
                return ps_stage.tile([P, n], F32, tag="ps1", name="pj")

            def project_v(st, tile_fn=None):
                # V natural: V[s, e] = sum_d xT[d, s] * WvT[d, e]
                pv = (tile_fn or proj_tile)()
                for dt_ in range(DT):
                    nc.tensor.matmul(
                        pv[:, 0:D],
                        xT[:, dt_, st * P:(st + 1) * P],
                        wvT[:, dt_, :],
                        start=(dt_ == 0),
                        stop=(dt_ == DT - 1),
                    )
                nc.vector.tensor_copy(vA[:, st, 0:D], pv[:, 0:D])
                nc.vector.tensor_copy(vA[:, st, D:D + 2], ones_c)

            def project_m():
                # M[d1, d2] = sum_e Wq[e, d1] * Wk[e, d2]
                for d1t in range(DT):
                    pm = proj_tile()
                    for et in range(DT):
                        nc.tensor.matmul(
                            pm[:, 0:D],
                            wqN[:, et, d1t * P:(d1t + 1) * P],
                            wkN[:, et, :],
                            start=(et == 0),
                            stop=(et == DT - 1),
                        )
                    nc.vector.tensor_copy(mSB[:, d1t, :], pm[:, 0:D])

            def project_g(qc, tile_fn=None):
                # GT[d2, s] = sum_d1 M[d1, d2] * xT[d1, s]
                for d2t in range(DT):
                    pp = (tile_fn or proj_tile)()
                    for d1t in range(DT):
                        nc.tensor.matmul(
                            pp,
                            mSB[:, d1t, d2t * P:(d2t + 1) * P],
                            xT[:, d1t, qc * QC:(qc + 1) * QC],
                            start=(d1t == 0),
                            stop=(d1t == DT - 1),
                        )
                    nc.vector.tensor_copy(
                        gT[:, d2t, qc * QC:(qc + 1) * QC], pp
                    )

            # Minimal pre-attention projections: V0-3 (wv + x0), M (wq/wk),
            # GT0 — attention chunk 0 starts right after; the remaining V
            # rows and GT chunks are interleaved into the attention stream
            # below, hidden under its matmuls while x1-3 stream in.
            def filler():
                nc.tensor.matmul(
                    pw, warm_r[:, 0:P], warm_r,
                    start=True, stop=True,
                )

            for st in range(4):
                project_v(st)
            # release the x1-3 loads: these one-element writes land on the
            # DVE stream after the vA0-3 evacuations, and the x-chunk DMAs
            # wait on them (WAW) before entering the rings.
            for qc in range(1, NQ):
                nc.vector.tensor_copy(
                    xT[:, 0, qc * QC:qc * QC + 1], ones_c[:, 0:1]
                )
            dma_x_rest()
            filler()
            project_m()
            project_g(0)

            # ---- attention, one 512-wide q chunk at a time ----------------
            for c in range(NQ):
                accs = [
                    ps_acc.tile([P, D + 2], F32, tag="acc", name="acc")
                    for _ in range(4)
                ]

                def emit_pv(kt_i, ex):
                    for qs in range(4):
                        nc.tensor.matmul(
                            accs[qs],
                            ex[:, qs * P:(qs + 1) * P],
                            vA[:, kt_i, :],
                            start=(kt_i == 0),
                            stop=(kt_i == ST - 1),
                        )

                pending = []
                for kt_i in range(ST):
                    # deferred projections, hidden under attention matmuls
                    if c == 0:
                        if kt_i == 3:
                            for st in range(4, 8):
                                project_v(st, proj_tile_stage)
                        elif kt_i == 5:
                            for st in range(8, 12):
                                project_v(st, proj_tile_stage)
                        elif kt_i == 8:
                            project_g(1, proj_tile_stage)
                        elif kt_i == 10:
                            for st in range(12, 16):
                                project_v(st, proj_tile_stage)
                        elif kt_i == 13:
                            project_g(2, proj_tile_stage)
                    elif c == 1 and kt_i == 2:
                        project_g(3, proj_tile_stage)
                    pa = ps_stage.tile([P, QC], F32, tag="ps1", name="pa")
                    for et in range(DT):
                        nc.tensor.matmul(
                            pa,
                            xT[:, et, kt_i * P:(kt_i + 1) * P],
                            gT[:, et, c * QC:(c + 1) * QC],
                            start=(et == 0),
                            stop=(et == DT - 1),
                        )
                    ex = ex_pool.tile([P, QC], BF16, tag="ex", name="ex")
                    nc.scalar.activation(
                        ex, pa, mybir.ActivationFunctionType.Exp
                    )
                    # software-pipeline PV two k-tiles behind the QK+exp so
                    # the PE never waits on a just-issued exp
                    pending.append((kt_i, ex))
                    if len(pending) > 2:
                        emit_pv(*pending.pop(0))
                for item in pending:
                    emit_pv(*item)

                # epilogue split across DVE and ACT so the tail chain halves;
                # all reciprocals first so the ACT-side muls never wait on a
                # reciprocal queued behind a DVE mul
                recs = []
                for qs in range(4):
                    rec = small_pool.tile([P, 1], F32, tag="rec", name="rec")
                    nc.vector.reciprocal(rec, accs[qs][:, D:D + 1])
                    recs.append(rec)
                for qs in range(4):
                    ob = ob_pool.tile([P, D], F32, tag="ob", name="ob")
                    qt_row = (c * 4 + qs) * P
                    if qs % 2:
                        nc.scalar.activation(
                            ob,
                            accs[qs][:, 0:D],
                            mybir.ActivationFunctionType.Copy,
                            scale=recs[qs],
                        )
                        nc.scalar.dma_start(
                            out=out[qt_row:qt_row + P, :], in_=ob
                        )
                    else:
                        nc.vector.tensor_scalar_mul(
                            ob, accs[qs][:, 0:D], recs[qs]
                        )
                        nc.sync.dma_start(
                            out=out[qt_row:qt_row + P, :], in_=ob
                        )

    nc.compile()
    return nc


_NC = None
_FAST = None


def _get_nc():
    global _NC
    if _NC is None:
        _NC = _build()
    return _NC


def _fast_runner():
    """Build (once) a jitted shard_map callable over the 8 cores."""
    global _FAST
    if _FAST is not None:
        return _FAST
    import jax
    from jax.experimental.shard_map import shard_map
    from jax.sharding import Mesh, PartitionSpec

    from concourse import bass2jax

    nc = _get_nc()
    bass2jax.install_neuronx_cc_hook()

    in_names = ["xt", "wq", "wk", "wvt"]
    out_aval = jax.core.ShapedArray((S, D), np.float32)

    def _body(*args):
        operands = list(args)
        operands.append(bass2jax.partition_id_tensor())
        outs = bass2jax._bass_exec_p.bind(
            *operands,
            out_avals=(out_aval,),
            in_names=tuple(in_names) + ("out", "partition_id"),
            out_names=("out",),
            lowering_input_output_aliases=(),
            sim_require_finite=True,
            sim_require_nnan=True,
            nc=nc,
        )
        return tuple(outs)

    devices = jax.devices()[:NB]
    mesh = Mesh(np.asarray(devices), ("core",))
    n_in = len(in_names) + 1  # + donated zero output
    fn = jax.jit(
        shard_map(
            _body,
            mesh=mesh,
            in_specs=(PartitionSpec("core"),) * n_in,
            out_specs=(PartitionSpec("core"),),
            check_rep=False,
        ),
        donate_argnums=(n_in - 1,),
        keep_unused=True,
    )
    _FAST = fn
    return fn


def _marshal(att_input, Wq, Wk, Wv):
    att_input = np.asarray(att_input, dtype=np.float32)
    # pre-transposed per-core x; Wv transposed; Wq/Wk natural (layout only)
    xts = np.ascontiguousarray(att_input.transpose(0, 2, 1))  # [NB, D, S]
    wq = np.ascontiguousarray(np.asarray(Wq, dtype=np.float32))
    wk = np.ascontiguousarray(np.asarray(Wk, dtype=np.float32))
    wvt = np.ascontiguousarray(np.asarray(Wv, dtype=np.float32).T)
    return xts, (wq, wk, wvt)


def run(att_input, Wq, Wk, Wv, trace=False):
    xts, wts = _marshal(att_input, Wq, Wk, Wv)
    if trace:
        in_maps = [
            {"xt": xts[b], "wq": wts[0], "wk": wts[1], "wvt": wts[2]}
            for b in range(NB)
        ]
        res = bass_utils.run_bass_kernel_spmd(
            _get_nc(), in_maps, core_ids=list(range(NB)), trace=True
        )
        out = np.stack([res.results[b]["out"] for b in range(NB)], axis=0)
        return out.astype(np.float32, copy=False), res

    try:
        fn = _fast_runner()
        xs = xts.reshape(NB * D, S)
        ws = [np.concatenate([w] * NB, axis=0) for w in wts]
        zeros = np.zeros((NB * S, D), np.float32)
        (out,) = fn(xs, *ws, zeros)
        out = np.asarray(out)
    except Exception:
        # fallback: the stock SPMD runner (re-jits per call, same NEFF)
        in_maps = [
            {"xt": xts[b], "wq": wts[0], "wk": wts[1], "wvt": wts[2]}
            for b in range(NB)
        ]
        res = bass_utils.run_bass_kernel_spmd(
            _get_nc(), in_maps, core_ids=list(range(NB))
        )
        out = np.stack([res.results[b]["out"] for b in range(NB)], axis=0)
    return out.reshape(NB, S, D).astype(np.float32, copy=False), None


def kernel(att_input, Wq, Wk, Wv):
    out, _ = run(att_input, Wq, Wk, Wv)
    return out
